# revision 5
# baseline (speedup 1.0000x reference)
"""Trainium2 Bass kernel for nn_AutoShiftsAug.

The reference op reduces to a per-batch constant 2D translation with bilinear
resampling over a replicate-padded, zero-extended image:

    out[b,c,i,j] = sum_{ty,tx in {0,1}} wy[b,ty,i] * wx[b,tx]
                   * XPZ[b, c, floor(gy_i)+ty, floor(gx_j)+tx]

gy_i ~= i + dy_b, gx_j ~= j + dx_b; XPZ is x edge-padded by 4, zero-extended.
All tap/weight data depends only on the tiny inputs (mean/var/eps/noise) and
is computed on host; the heavy data movement runs on 8 NeuronCores,
batch-sharded (16 batches per core).

Hardware-validated indirect-DMA contract: ONE index per partition ([P,1]
offset AP), each index fetching one contiguous run into that partition.

Per core, per batch:
  phase 1: two indirect row-gathers of x (vertical taps, all 9 channels per
           row contiguous thanks to a host-side [b,i,c,w] transpose of x)
           -> per-row weighted vertical blend -> z tile with padded columns
           (replicate edges + zero margins) -> plain store to DRAM scratch
           z2 (rows of 9*152).
  phase 2: one indirect gather per batch whose per-partition start offset
           encodes the per-batch column shift X0 -> static 2-tap column
           blend -> plain store (out in [b,i,c,w]; host transposes back).
"""

import numpy as np

PAD = 4
H = 128
HP = H + 2 * PAD  # 136
NCH = 9
NB_TOT = 128
NCORES = 8
NB = NB_TOT // NCORES  # batches per core
WS = 152  # per-channel padded z row: col t <-> padded col p = t - 8
ZROW = NCH * WS  # 1368, one z row per (b, i)
RUN2 = (NCH - 1) * WS + H + 2  # 1346: phase-2 contiguous run per partition


# ----------------------------------------------------------------------------
# host-side parameter computation (fp32, mirroring the jax reference math)
# ----------------------------------------------------------------------------
def _host_params(mean, var, eps, noise):
    f32 = np.float32
    mean = np.asarray(mean, f32)
    var = np.asarray(var, f32)
    eps = np.asarray(eps, f32)
    noise = np.asarray(noise, f32)

    bound = f32(2.0 * (2 * PAD + 1) / HP)
    m = np.clip(mean, f32(1e-6), bound).astype(f32)
    s = np.clip(var, f32(1e-6), None).astype(f32)
    shift = np.clip(m + s * eps, f32(0.0), bound).astype(f32)  # (2,)

    ar = np.linspace(f32(-1.0 + 1.0 / HP), f32(1.0 - 1.0 / HP), HP, dtype=f32)[:H]

    # exact fp32 sampling coordinate (padded-image space) per batch/index:
    # g[b,k] = (ar[k] + shift[a] + noise[b,a] + 1) * (HP/2) - 0.5
    def coords(a):
        g = (
            ar[None, :] + shift[a] + noise[:, 0, 0, a][:, None] + f32(1.0)
        ) * f32(HP * 0.5) - f32(0.5)
        return g.astype(f32)

    gx = coords(0)  # column axis (varies along j)
    gy = coords(1)  # row axis (varies along i)

    # vertical: per-row exact taps/weights
    a0 = np.floor(gy).astype(np.int64)
    fy = (gy - a0).astype(f32)
    v0 = ((a0 >= 0) & (a0 < HP)).astype(f32)
    v1 = ((a0 + 1 >= 0) & (a0 + 1 < HP)).astype(f32)
    wy0 = ((f32(1.0) - fy) * v0).astype(f32)
    wy1 = (fy * v1).astype(f32)
    r0 = np.clip(a0 - PAD, 0, H - 1).astype(np.int32)
    r1 = np.clip(a0 + 1 - PAD, 0, H - 1).astype(np.int32)

    # horizontal: per-batch uniform tap/weight
    d = gx - np.arange(H, dtype=f32)[None, :]
    dm = d.mean(axis=1, dtype=np.float64).astype(f32)
    X0 = np.clip(np.floor(dm).astype(np.int64), -PAD, 3 * PAD).astype(np.int32)
    fx = (dm - X0).astype(f32)

    return r0, r1, wy0, wy1, X0, fx


def _core_inputs(x, r0, r1, wy0, wy1, X0, fx, k):
    """Per-core input arrays for core k. x is the full [128,9,128,128] array."""
    b0 = k * NB
    bs = slice(b0, b0 + NB)
    # [b, i, c, w] layout: each (b, i) row holds all 9 channels contiguously
    xs = np.ascontiguousarray(x[bs].transpose(0, 2, 1, 3)).reshape(NB * H, NCH * H)

    iy = np.zeros((NB, 2, H, 1), np.int32)
    wy = np.zeros((NB, H, 2), np.float32)
    iz = np.zeros((NB, H, 1), np.int32)
    wx = np.zeros((NB, H, 2), np.float32)
    i = np.arange(H, dtype=np.int32)
    for bl in range(NB):
        bg = b0 + bl
        iy[bl, 0, :, 0] = bl * H + r0[bg]
        iy[bl, 1, :, 0] = bl * H + r1[bg]
        wy[bl, :, 0] = wy0[bg]
        wy[bl, :, 1] = wy1[bg]
        iz[bl, :, 0] = (bl * H + i) * ZROW + 8 + int(X0[bg])
        wx[bl, :, 0] = 1.0 - fx[bg]
        wx[bl, :, 1] = fx[bg]
    return {"x": xs, "iy": iy, "wy": wy, "iz": iz, "wx": wx}


# ----------------------------------------------------------------------------
# bass program
# ----------------------------------------------------------------------------
_PROG_CACHE = {}


def _build_program():
    import concourse.bacc as bacc
    import concourse.tile as tile
    import concourse.mybir as mybir
    import concourse.bass as bass
    from concourse.tile import add_dep_helper

    f32 = mybir.dt.float32
    i32 = mybir.dt.int32
    mult = mybir.AluOpType.mult
    add = mybir.AluOpType.add

    nc = bacc.Bacc("TRN2", target_bir_lowering=False, num_devices=NCORES, debug=False)

    xd = nc.dram_tensor("x", [NB * H, NCH * H], f32, kind="ExternalInput")
    iyd = nc.dram_tensor("iy", [NB, 2, H, 1], i32, kind="ExternalInput")
    wyd = nc.dram_tensor("wy", [NB, H, 2], f32, kind="ExternalInput")
    izd = nc.dram_tensor("iz", [NB, H, 1], i32, kind="ExternalInput")
    wxd = nc.dram_tensor("wx", [NB, H, 2], f32, kind="ExternalInput")
    zd = nc.dram_tensor("z", [NB * H * ZROW, 1], f32)
    outd = nc.dram_tensor("out", [NB, H, NCH, H], f32, kind="ExternalOutput")

    zd2 = zd.ap().rearrange("(r w) one -> r (w one)", w=ZROW)  # [NB*H, ZROW]

    with tile.TileContext(nc) as tc:
        with tc.tile_pool(name="p", bufs=3) as pool:
            for b in range(NB):
                # ---- param loads ----
                iy0 = pool.tile([H, 1], i32, tag="iy0")
                nc.sync.dma_start(iy0[:], iyd.ap()[b, 0])
                iy1 = pool.tile([H, 1], i32, tag="iy1")
                nc.sync.dma_start(iy1[:], iyd.ap()[b, 1])
                wyt = pool.tile([H, 2], f32, tag="wyt")
                nc.sync.dma_start(wyt[:], wyd.ap()[b])
                izt = pool.tile([H, 1], i32, tag="izt")
                nc.sync.dma_start(izt[:], izd.ap()[b])
                wxt = pool.tile([H, 2], f32, tag="wxt")
                nc.sync.dma_start(wxt[:], wxd.ap()[b])

                # ---- phase 1: vertical taps + blend ----
                # NB: indirect-DMA dest APs must be flat [P, run] — 3-dim
                # dest APs lower incorrectly on HW.
                g0 = pool.tile([H, NCH * H], f32, tag="g0")
                nc.gpsimd.indirect_dma_start(
                    out=g0[:],
                    out_offset=None,
                    in_=xd.ap()[:, :],
                    in_offset=bass.IndirectOffsetOnAxis(ap=iy0[:, :], axis=0),
                )
                g1 = pool.tile([H, NCH * H], f32, tag="g1")
                nc.gpsimd.indirect_dma_start(
                    out=g1[:],
                    out_offset=None,
                    in_=xd.ap()[:, :],
                    in_offset=bass.IndirectOffsetOnAxis(ap=iy1[:, :], axis=0),
                )
                g0v = g0[:].rearrange("p (c w) -> p c w", c=NCH)
                g1v = g1[:].rearrange("p (c w) -> p c w", c=NCH)

                t1 = pool.tile([H, NCH, H], f32, tag="t1")
                nc.scalar.mul(t1[:], g1v, wyt[:, 1:2])

                zt = pool.tile([H, NCH, WS], f32, tag="zt")
                nc.gpsimd.memset(zt[:, :, 0:8], 0.0)
                nc.gpsimd.memset(zt[:, :, 144:WS], 0.0)
                nc.vector.scalar_tensor_tensor(
                    out=zt[:, :, 12:140],
                    in0=g0v,
                    scalar=wyt[:, 0:1],
                    in1=t1[:],
                    op0=mult,
                    op1=add,
                )
                # replicate-pad columns: padded col p in [0,4) <- orig col 0,
                # [132,136) <- orig col 127  (t = p + 8)
                nc.vector.tensor_copy(
                    zt[:, :, 8:12], zt[:, :, 12:13].to_broadcast([H, NCH, PAD])
                )
                nc.vector.tensor_copy(
                    zt[:, :, 140:144], zt[:, :, 139:140].to_broadcast([H, NCH, PAD])
                )

                st = nc.sync.dma_start(
                    zd2[b * H : (b + 1) * H, :],
                    zt[:].rearrange("p c w -> p (c w)"),
                )

                # ---- phase 2: shifted gather + horizontal blend ----
                zw = pool.tile([H, NCH, WS], f32, tag="zw")
                g2 = nc.gpsimd.indirect_dma_start(
                    out=zw[:].rearrange("p c w -> p (c w)")[:, 0:RUN2],
                    out_offset=None,
                    in_=zd.ap()[:, :],
                    in_offset=bass.IndirectOffsetOnAxis(ap=izt[:, :], axis=0),
                )
                add_dep_helper(
                    g2.ins,
                    st.ins,
                    reason="phase2 gather reads z written by phase1 store",
                )

                t2 = pool.tile([H, NCH, H], f32, tag="t2")
                nc.scalar.mul(t2[:], zw[:, :, 1 : H + 1], wxt[:, 1:2])
                ot = pool.tile([H, NCH, H], f32, tag="ot")
                nc.vector.scalar_tensor_tensor(
                    out=ot[:],
                    in0=zw[:, :, 0:H],
                    scalar=wxt[:, 0:1],
                    in1=t2[:],
                    op0=mult,
                    op1=add,
                )
                nc.sync.dma_start(outd.ap()[b], ot[:])

    nc.compile()
    return nc


def _get_program():
    if "nc" not in _PROG_CACHE:
        _PROG_CACHE["nc"] = _build_program()
    return _PROG_CACHE["nc"]


# ----------------------------------------------------------------------------
# entry point
# ----------------------------------------------------------------------------
def kernel(x, mean, var, eps, noise):
    from concourse.bass_utils import run_bass_kernel_spmd

    x = np.ascontiguousarray(np.asarray(x, np.float32))
    params = _host_params(mean, var, eps, noise)
    in_maps = [_core_inputs(x, *params, k) for k in range(NCORES)]

    nc = _get_program()
    res = run_bass_kernel_spmd(nc, in_maps, core_ids=list(range(NCORES)))
    out = np.concatenate(
        [res.results[k]["out"].transpose(0, 2, 1, 3) for k in range(NCORES)], axis=0
    )
    return np.ascontiguousarray(out.astype(np.float32))


# revision 7
# speedup vs baseline: 1.9886x; 1.9886x over previous
"""Trainium2 Bass kernel for nn_AutoShiftsAug.

The reference op reduces to a per-batch constant 2D translation with bilinear
resampling over a replicate-padded, zero-extended image:

    out[b,c,i,j] = w00*XPZ[b,c,y0_i,x0_j]   + w01*XPZ[b,c,y0_i,x0_j+1]
                 + w10*XPZ[b,c,y0_i+1,x0_j] + w11*XPZ[b,c,y0_i+1,x0_j+1]

with y0_i = floor(gy_i) (per-row exact), x0_j = j + X0 (per-batch uniform),
XPZ = x edge-padded by 4 then zero-extended.  All tap/weight data depends
only on the tiny inputs (mean/var/eps/noise) and is computed on host.

Layout trick: the host ships x column-padded and channel-flattened —
xp[b*H + r, c*152 + t] where col t <-> padded col p = t - 8 (replicate
edges + zero margins baked in).  A single indirect-DMA gather per
(batch, row-tap) with one index per partition then fetches, for output row
i, the contiguous run starting at (row base + 8 + X0): after the gather,
channel c's output column j sits at flat position c*152 + j for tap x0,
c*152 + j + 1 for tap x0+1 — the whole 2D shift reduces to static slices.
The 4-tap blend uses per-partition premultiplied weights.

Hardware-validated indirect-DMA contract: ONE index per partition ([P,1]
offset AP), flat 2-D dest AP, contiguous run per index.
"""

import numpy as np

PAD = 4
H = 128
HP = H + 2 * PAD  # 136
NCH = 9
NB_TOT = 128
NCORES = 8
NB = NB_TOT // NCORES  # batches per core
WS = 152  # padded channel row width; col t <-> padded col p = t - 8
XROW = NCH * WS  # 1368
RUN = (NCH - 1) * WS + H + 2  # 1346 contiguous elements per gathered row


# ----------------------------------------------------------------------------
# host-side parameter computation (fp32, mirroring the jax reference math)
# ----------------------------------------------------------------------------
def _host_params(mean, var, eps, noise):
    f32 = np.float32
    mean = np.asarray(mean, f32)
    var = np.asarray(var, f32)
    eps = np.asarray(eps, f32)
    noise = np.asarray(noise, f32)

    bound = f32(2.0 * (2 * PAD + 1) / HP)
    m = np.clip(mean, f32(1e-6), bound).astype(f32)
    s = np.clip(var, f32(1e-6), None).astype(f32)
    shift = np.clip(m + s * eps, f32(0.0), bound).astype(f32)  # (2,)

    ar = np.linspace(f32(-1.0 + 1.0 / HP), f32(1.0 - 1.0 / HP), HP, dtype=f32)[:H]

    def coords(a):
        g = (
            ar[None, :] + shift[a] + noise[:, 0, 0, a][:, None] + f32(1.0)
        ) * f32(HP * 0.5) - f32(0.5)
        return g.astype(f32)

    gx = coords(0)  # column axis (varies along j)
    gy = coords(1)  # row axis (varies along i)

    # vertical: per-row exact taps/weights
    a0 = np.floor(gy).astype(np.int64)
    fy = (gy - a0).astype(f32)
    v0 = ((a0 >= 0) & (a0 < HP)).astype(f32)
    v1 = ((a0 + 1 >= 0) & (a0 + 1 < HP)).astype(f32)
    wy0 = ((f32(1.0) - fy) * v0).astype(f32)
    wy1 = (fy * v1).astype(f32)
    r0 = np.clip(a0 - PAD, 0, H - 1).astype(np.int32)
    r1 = np.clip(a0 + 1 - PAD, 0, H - 1).astype(np.int32)

    # horizontal: per-batch uniform tap/weight
    d = gx - np.arange(H, dtype=f32)[None, :]
    dm = d.mean(axis=1, dtype=np.float64).astype(f32)
    X0 = np.clip(np.floor(dm).astype(np.int64), -PAD, 3 * PAD).astype(np.int32)
    fx = (dm - X0).astype(f32)

    return r0, r1, wy0, wy1, X0, fx


def _pad_cols(xs):
    """[N, H(rows), NCH, H(cols)] -> [N, H, NCH, WS] with padded columns."""
    n = xs.shape[0]
    xp = np.zeros((n, H, NCH, WS), np.float32)
    xp[..., 12:140] = xs
    xp[..., 8:12] = xs[..., 0:1]
    xp[..., 140:144] = xs[..., H - 1 : H]
    return xp


def _core_inputs(x, r0, r1, wy0, wy1, X0, fx, k):
    """Per-core input arrays for core k. x is the full [128,9,128,128] array."""
    b0 = k * NB
    bs = slice(b0, b0 + NB)
    xp = _pad_cols(x[bs].transpose(0, 2, 1, 3)).reshape(NB * H * XROW, 1)

    iy = np.zeros((NB, 2, H, 1), np.int32)
    w4 = np.zeros((NB, H, 4), np.float32)
    for bl in range(NB):
        bg = b0 + bl
        iy[bl, 0, :, 0] = (bl * H + r0[bg]) * XROW + 8 + int(X0[bg])
        iy[bl, 1, :, 0] = (bl * H + r1[bg]) * XROW + 8 + int(X0[bg])
        wx0 = np.float32(1.0) - fx[bg]
        wx1 = fx[bg]
        w4[bl, :, 0] = wy0[bg] * wx0
        w4[bl, :, 1] = wy0[bg] * wx1
        w4[bl, :, 2] = wy1[bg] * wx0
        w4[bl, :, 3] = wy1[bg] * wx1
    return {"x": xp, "iy": iy, "w4": w4}


# ----------------------------------------------------------------------------
# bass program
# ----------------------------------------------------------------------------
_PROG_CACHE = {}


def _build_program():
    import concourse.bacc as bacc
    import concourse.tile as tile
    import concourse.mybir as mybir
    import concourse.bass as bass

    f32 = mybir.dt.float32
    i32 = mybir.dt.int32
    mult = mybir.AluOpType.mult
    add = mybir.AluOpType.add

    nc = bacc.Bacc("TRN2", target_bir_lowering=False, num_devices=NCORES, debug=False)

    xd = nc.dram_tensor("x", [NB * H * XROW, 1], f32, kind="ExternalInput")
    iyd = nc.dram_tensor("iy", [NB, 2, H, 1], i32, kind="ExternalInput")
    w4d = nc.dram_tensor("w4", [NB, H, 4], f32, kind="ExternalInput")
    outd = nc.dram_tensor("out", [NB, H, NCH, H], f32, kind="ExternalOutput")

    with tile.TileContext(nc) as tc:
        with tc.tile_pool(name="p", bufs=3) as pool:
            for b in range(NB):
                iy0 = pool.tile([H, 1], i32, tag="iy0")
                nc.sync.dma_start(iy0[:], iyd.ap()[b, 0])
                iy1 = pool.tile([H, 1], i32, tag="iy1")
                nc.sync.dma_start(iy1[:], iyd.ap()[b, 1])
                wt = pool.tile([H, 4], f32, tag="wt")
                nc.sync.dma_start(wt[:], w4d.ap()[b])

                g0 = pool.tile([H, XROW], f32, tag="g0")
                nc.gpsimd.indirect_dma_start(
                    out=g0[:, 0:RUN],
                    out_offset=None,
                    in_=xd.ap()[:, :],
                    in_offset=bass.IndirectOffsetOnAxis(ap=iy0[:, :], axis=0),
                )
                g1 = pool.tile([H, XROW], f32, tag="g1")
                nc.gpsimd.indirect_dma_start(
                    out=g1[:, 0:RUN],
                    out_offset=None,
                    in_=xd.ap()[:, :],
                    in_offset=bass.IndirectOffsetOnAxis(ap=iy1[:, :], axis=0),
                )
                g0v = g0[:].rearrange("p (c w) -> p c w", w=WS)
                g1v = g1[:].rearrange("p (c w) -> p c w", w=WS)
                A = g0v[:, :, 0:H]
                B = g0v[:, :, 1 : H + 1]
                C = g1v[:, :, 0:H]
                D = g1v[:, :, 1 : H + 1]

                p1 = pool.tile([H, NCH, H], f32, tag="p1")
                nc.scalar.mul(p1[:], A, wt[:, 0:1])
                p2 = pool.tile([H, NCH, H], f32, tag="p2")
                nc.scalar.mul(p2[:], C, wt[:, 2:3])
                q1 = pool.tile([H, NCH, H], f32, tag="q1")
                nc.vector.scalar_tensor_tensor(
                    out=q1[:], in0=B, scalar=wt[:, 1:2], in1=p1[:], op0=mult, op1=add
                )
                q2 = pool.tile([H, NCH, H], f32, tag="q2")
                nc.vector.scalar_tensor_tensor(
                    out=q2[:], in0=D, scalar=wt[:, 3:4], in1=p2[:], op0=mult, op1=add
                )
                ot = pool.tile([H, NCH, H], f32, tag="ot")
                nc.vector.tensor_add(ot[:], q1[:], q2[:])
                nc.sync.dma_start(outd.ap()[b], ot[:])

    nc.compile()
    return nc


def _get_program():
    if "nc" not in _PROG_CACHE:
        _PROG_CACHE["nc"] = _build_program()
    return _PROG_CACHE["nc"]


# ----------------------------------------------------------------------------
# entry point
# ----------------------------------------------------------------------------
def kernel(x, mean, var, eps, noise):
    from concourse.bass_utils import run_bass_kernel_spmd

    x = np.ascontiguousarray(np.asarray(x, np.float32))
    params = _host_params(mean, var, eps, noise)
    in_maps = [_core_inputs(x, *params, k) for k in range(NCORES)]

    nc = _get_program()
    res = run_bass_kernel_spmd(nc, in_maps, core_ids=list(range(NCORES)))
    out = np.concatenate(
        [res.results[k]["out"].transpose(0, 2, 1, 3) for k in range(NCORES)], axis=0
    )
    return np.ascontiguousarray(out.astype(np.float32))


# revision 11
# speedup vs baseline: 2.3659x; 1.1897x over previous
"""Trainium2 Bass kernel for nn_AutoShiftsAug.

The reference op reduces to a per-batch constant 2D translation with bilinear
resampling over a replicate-padded, zero-extended image:

    out[b,c,i,j] = sum_{ty,tx} wy[b,ty,i] * wx[b,tx]
                   * XPZ[b, c, ytap(b,ty,i), j + X0_b + tx]

with per-row-exact vertical taps ytap and a per-batch uniform horizontal tap
X0_b.  All tap/weight data depends only on the tiny inputs
(mean/var/eps/noise) and is computed on host; batch-sharded across 8 cores.

Device pipeline per batch (all APs static; per-batch variation rides in
input data only):
  1. one indirect-DMA gather: partition r <- host-column-padded x row
     (b, r), 1346 contiguous floats starting at column offset 8+X0_b.
     After the gather, channel c's output column j sits at flat position
     c*152 + j (tap x0) / c*152 + j + 1 (tap x0+1).
  2. TensorE: z = Wy @ G — the banded per-batch vertical-blend matrix
     (host-built, exact weights incl. replicate-clamp merging and
     zero-validity) applied as 3 matmul chunks into one PSUM tile.
  3. ScalarE/VectorE: out = wx0 * z[:, :, 0:128] + wx1 * z[:, :, 1:129].
  4. store (out in [b, i, c, w]; host transposes back).

Hardware-validated indirect-DMA contract: ONE index per partition ([P,1]
offset AP), flat 2-D dest AP, contiguous run per index.
"""

import numpy as np

PAD = 4
H = 128
HP = H + 2 * PAD  # 136
NCH = 9
NB_TOT = 128
NCORES = 8
NB = NB_TOT // NCORES  # batches per core
WS = 152  # padded channel row width; col t <-> padded col p = t - 8
XROW = NCH * WS  # 1368
RUN = (NCH - 1) * WS + H + 2  # 1346 contiguous elements per gathered row
MMCHUNK = 512  # fp32 matmul moving-dim limit


# ----------------------------------------------------------------------------
# host-side parameter computation (fp32, mirroring the jax reference math)
# ----------------------------------------------------------------------------
def _host_params(mean, var, eps, noise):
    f32 = np.float32
    mean = np.asarray(mean, f32)
    var = np.asarray(var, f32)
    eps = np.asarray(eps, f32)
    noise = np.asarray(noise, f32)

    bound = f32(2.0 * (2 * PAD + 1) / HP)
    m = np.clip(mean, f32(1e-6), bound).astype(f32)
    s = np.clip(var, f32(1e-6), None).astype(f32)
    shift = np.clip(m + s * eps, f32(0.0), bound).astype(f32)  # (2,)

    ar = np.linspace(f32(-1.0 + 1.0 / HP), f32(1.0 - 1.0 / HP), HP, dtype=f32)[:H]

    def coords(a):
        g = (
            ar[None, :] + shift[a] + noise[:, 0, 0, a][:, None] + f32(1.0)
        ) * f32(HP * 0.5) - f32(0.5)
        return g.astype(f32)

    gx = coords(0)  # column axis (varies along j)
    gy = coords(1)  # row axis (varies along i)

    # vertical: per-row exact taps/weights
    a0 = np.floor(gy).astype(np.int64)
    fy = (gy - a0).astype(f32)
    v0 = ((a0 >= 0) & (a0 < HP)).astype(f32)
    v1 = ((a0 + 1 >= 0) & (a0 + 1 < HP)).astype(f32)
    wy0 = ((f32(1.0) - fy) * v0).astype(f32)
    wy1 = (fy * v1).astype(f32)
    r0 = np.clip(a0 - PAD, 0, H - 1).astype(np.int32)
    r1 = np.clip(a0 + 1 - PAD, 0, H - 1).astype(np.int32)

    # horizontal: per-batch uniform tap/weight
    d = gx - np.arange(H, dtype=f32)[None, :]
    dm = d.mean(axis=1, dtype=np.float64).astype(f32)
    X0 = np.clip(np.floor(dm).astype(np.int64), -PAD, 3 * PAD).astype(np.int32)
    fx = (dm - X0).astype(f32)

    return r0, r1, wy0, wy1, X0, fx


def _pad_cols(xs):
    """[N, H(rows), NCH, H(cols)] -> [N, H, NCH, WS] with padded columns."""
    n = xs.shape[0]
    xp = np.zeros((n, H, NCH, WS), np.float32)
    xp[..., 12:140] = xs
    xp[..., 8:12] = xs[..., 0:1]
    xp[..., 140:144] = xs[..., H - 1 : H]
    return xp


def _core_inputs(x, r0, r1, wy0, wy1, X0, fx, k):
    """Per-core input arrays for core k. x is the full [128,9,128,128] array."""
    b0 = k * NB
    bs = slice(b0, b0 + NB)
    xp = _pad_cols(x[bs].transpose(0, 2, 1, 3)).reshape(NB * H * XROW, 1)

    iy = np.zeros((NB, H, 1), np.int32)
    wxp = np.zeros((NB, H, 2), np.float32)
    wyT = np.zeros((NB, H, H), np.float32)
    r = np.arange(H, dtype=np.int64)
    for bl in range(NB):
        bg = b0 + bl
        iy[bl, :, 0] = (bl * H + r) * XROW + 8 + int(X0[bg])
        wxp[bl, :, 0] = 1.0 - fx[bg]
        wxp[bl, :, 1] = fx[bg]
        # Wy[i, rr]: vertical blend matrix; WyT = Wy.T shipped for lhsT
        Wy = np.zeros((H, H), np.float32)
        np.add.at(Wy, (r, r0[bg]), wy0[bg])
        np.add.at(Wy, (r, r1[bg]), wy1[bg])
        wyT[bl] = Wy.T
    return {"x": xp, "iy": iy, "wxp": wxp, "wyT": wyT}


# ----------------------------------------------------------------------------
# bass program
# ----------------------------------------------------------------------------
_PROG_CACHE = {}


def _build_program():
    import concourse.bacc as bacc
    import concourse.tile as tile
    import concourse.mybir as mybir
    import concourse.bass as bass

    f32 = mybir.dt.float32
    i32 = mybir.dt.int32
    mult = mybir.AluOpType.mult
    add = mybir.AluOpType.add

    nc = bacc.Bacc("TRN2", target_bir_lowering=False, num_devices=NCORES, debug=False)

    xd = nc.dram_tensor("x", [NB * H * XROW, 1], f32, kind="ExternalInput")
    iyd = nc.dram_tensor("iy", [NB, H, 1], i32, kind="ExternalInput")
    wxd = nc.dram_tensor("wxp", [NB, H, 2], f32, kind="ExternalInput")
    wyd = nc.dram_tensor("wyT", [NB, H, H], f32, kind="ExternalInput")
    outd = nc.dram_tensor("out", [NB, H, NCH, H], f32, kind="ExternalOutput")

    with tile.TileContext(nc) as tc:
        with (
            tc.tile_pool(name="p", bufs=3) as pool,
            tc.tile_pool(name="ps", bufs=2, space="PSUM") as psum,
        ):
            for b in range(NB):
                iyt = pool.tile([H, 1], i32, tag="iyt")
                nc.sync.dma_start(iyt[:], iyd.ap()[b])
                wxt = pool.tile([H, 2], f32, tag="wxt")
                nc.sync.dma_start(wxt[:], wxd.ap()[b])
                wyt = pool.tile([H, H], f32, tag="wyt")
                nc.sync.dma_start(wyt[:], wyd.ap()[b])

                g = pool.tile([H, XROW], f32, tag="g")
                nc.gpsimd.indirect_dma_start(
                    out=g[:, 0:RUN],
                    out_offset=None,
                    in_=xd.ap()[:, :],
                    in_offset=bass.IndirectOffsetOnAxis(ap=iyt[:, :], axis=0),
                )

                z = psum.tile([H, XROW], f32, tag="z")
                for c0 in range(0, RUN, MMCHUNK):
                    c1 = min(c0 + MMCHUNK, RUN)
                    nc.tensor.matmul(
                        out=z[:, c0:c1],
                        lhsT=wyt[:],
                        rhs=g[:, c0:c1],
                        start=True,
                        stop=True,
                    )

                zv = z[:].rearrange("p (c w) -> p c w", w=WS)
                p1 = pool.tile([H, NCH, H], f32, tag="p1")
                nc.scalar.mul(p1[:], zv[:, :, 0:H], wxt[:, 0:1])
                ot = pool.tile([H, NCH, H], f32, tag="ot")
                nc.vector.scalar_tensor_tensor(
                    out=ot[:],
                    in0=zv[:, :, 1 : H + 1],
                    scalar=wxt[:, 1:2],
                    in1=p1[:],
                    op0=mult,
                    op1=add,
                )
                nc.sync.dma_start(outd.ap()[b], ot[:])

    nc.compile()
    return nc


def _get_program():
    if "nc" not in _PROG_CACHE:
        _PROG_CACHE["nc"] = _build_program()
    return _PROG_CACHE["nc"]


# ----------------------------------------------------------------------------
# entry point
# ----------------------------------------------------------------------------
def kernel(x, mean, var, eps, noise):
    from concourse.bass_utils import run_bass_kernel_spmd

    x = np.ascontiguousarray(np.asarray(x, np.float32))
    params = _host_params(mean, var, eps, noise)
    in_maps = [_core_inputs(x, *params, k) for k in range(NCORES)]

    nc = _get_program()
    res = run_bass_kernel_spmd(nc, in_maps, core_ids=list(range(NCORES)))
    out = np.concatenate(
        [res.results[k]["out"].transpose(0, 2, 1, 3) for k in range(NCORES)], axis=0
    )
    return np.ascontiguousarray(out.astype(np.float32))


# revision 13
# speedup vs baseline: 2.7517x; 1.1631x over previous
"""Trainium2 Bass kernel for nn_AutoShiftsAug.

The reference op reduces to a per-batch constant 2D translation with bilinear
resampling over a replicate-padded, zero-extended image:

    out[b,c,i,j] = sum_{ty,tx} wy[b,ty,i] * wx[b,tx]
                   * XPZ[b, c, ytap(b,ty,i), j + X0_b + tx]

with per-row-exact vertical taps ytap and a per-batch uniform horizontal tap
X0_b.  All tap/weight data depends only on the tiny inputs
(mean/var/eps/noise) and is computed on host; batch-sharded across 8 cores.

Device pipeline per batch (all APs static; per-batch variation rides in
input data only):
  1. one indirect-DMA gather: partition r <- host-column-padded x row
     (b, r), 1346 contiguous floats starting at column offset 8+X0_b.
     After the gather, channel c's output column j sits at flat position
     c*152 + j (tap x0) / c*152 + j + 1 (tap x0+1).
  2. TensorE: z = Wy @ G — the banded per-batch vertical-blend matrix
     (host-built, exact weights incl. replicate-clamp merging and
     zero-validity) applied as 3 matmul chunks into one PSUM tile.
  3. ScalarE/VectorE: out = wx0 * z[:, :, 0:128] + wx1 * z[:, :, 1:129].
  4. store (out in [b, i, c, w]; host transposes back).

Hardware-validated indirect-DMA contract: ONE index per partition ([P,1]
offset AP), flat 2-D dest AP, contiguous run per index.
"""

import numpy as np

PAD = 4
H = 128
HP = H + 2 * PAD  # 136
NCH = 9
NB_TOT = 128
NCORES = 8
NB = NB_TOT // NCORES  # batches per core
WS = 152  # padded channel row width; col t <-> padded col p = t - 8
XROW = NCH * WS  # 1368
RUN = (NCH - 1) * WS + H + 2  # 1346 contiguous elements per gathered row
MMCHUNK = 512  # fp32 matmul moving-dim limit


# ----------------------------------------------------------------------------
# host-side parameter computation (fp32, mirroring the jax reference math)
# ----------------------------------------------------------------------------
def _host_params(mean, var, eps, noise):
    f32 = np.float32
    mean = np.asarray(mean, f32)
    var = np.asarray(var, f32)
    eps = np.asarray(eps, f32)
    noise = np.asarray(noise, f32)

    bound = f32(2.0 * (2 * PAD + 1) / HP)
    m = np.clip(mean, f32(1e-6), bound).astype(f32)
    s = np.clip(var, f32(1e-6), None).astype(f32)
    shift = np.clip(m + s * eps, f32(0.0), bound).astype(f32)  # (2,)

    ar = np.linspace(f32(-1.0 + 1.0 / HP), f32(1.0 - 1.0 / HP), HP, dtype=f32)[:H]

    def coords(a):
        g = (
            ar[None, :] + shift[a] + noise[:, 0, 0, a][:, None] + f32(1.0)
        ) * f32(HP * 0.5) - f32(0.5)
        return g.astype(f32)

    gx = coords(0)  # column axis (varies along j)
    gy = coords(1)  # row axis (varies along i)

    # vertical: per-row exact taps/weights
    a0 = np.floor(gy).astype(np.int64)
    fy = (gy - a0).astype(f32)
    v0 = ((a0 >= 0) & (a0 < HP)).astype(f32)
    v1 = ((a0 + 1 >= 0) & (a0 + 1 < HP)).astype(f32)
    wy0 = ((f32(1.0) - fy) * v0).astype(f32)
    wy1 = (fy * v1).astype(f32)
    r0 = np.clip(a0 - PAD, 0, H - 1).astype(np.int32)
    r1 = np.clip(a0 + 1 - PAD, 0, H - 1).astype(np.int32)

    # horizontal: per-batch uniform tap/weight
    d = gx - np.arange(H, dtype=f32)[None, :]
    dm = d.mean(axis=1, dtype=np.float64).astype(f32)
    X0 = np.clip(np.floor(dm).astype(np.int64), -PAD, 3 * PAD).astype(np.int32)
    fx = (dm - X0).astype(f32)

    return r0, r1, wy0, wy1, X0, fx


def _pad_cols(xs):
    """[N, H(rows), NCH, H(cols)] -> [N, H, NCH, WS] with padded columns."""
    n = xs.shape[0]
    xp = np.zeros((n, H, NCH, WS), np.float32)
    xp[..., 12:140] = xs
    xp[..., 8:12] = xs[..., 0:1]
    xp[..., 140:144] = xs[..., H - 1 : H]
    return xp


def _core_inputs(x, r0, r1, wy0, wy1, X0, fx, k):
    """Per-core input arrays for core k. x is the full [128,9,128,128] array."""
    b0 = k * NB
    bs = slice(b0, b0 + NB)
    xp = _pad_cols(x[bs].transpose(0, 2, 1, 3)).reshape(NB * H * XROW, 1)

    # per-partition-major parameter blocks, loaded once for the whole core:
    # iy[p, b] gather index, wxp[p, 2*b:2*b+2] horizontal weights
    iy = np.zeros((H, NB), np.int32)
    wxp = np.zeros((H, 2 * NB), np.float32)
    wyT = np.zeros((NB, H, H), np.float32)
    r = np.arange(H, dtype=np.int64)
    for bl in range(NB):
        bg = b0 + bl
        iy[:, bl] = (bl * H + r) * XROW + 8 + int(X0[bg])
        wxp[:, 2 * bl] = 1.0 - fx[bg]
        wxp[:, 2 * bl + 1] = fx[bg]
        # Wy[i, rr]: vertical blend matrix; WyT = Wy.T shipped for lhsT
        Wy = np.zeros((H, H), np.float32)
        np.add.at(Wy, (r, r0[bg]), wy0[bg])
        np.add.at(Wy, (r, r1[bg]), wy1[bg])
        wyT[bl] = Wy.T
    return {"x": xp, "iy": iy, "wxp": wxp, "wyT": wyT}


# ----------------------------------------------------------------------------
# bass program
# ----------------------------------------------------------------------------
_PROG_CACHE = {}


def _build_program():
    import concourse.bacc as bacc
    import concourse.tile as tile
    import concourse.mybir as mybir
    import concourse.bass as bass

    f32 = mybir.dt.float32
    i32 = mybir.dt.int32
    mult = mybir.AluOpType.mult
    add = mybir.AluOpType.add

    nc = bacc.Bacc("TRN2", target_bir_lowering=False, num_devices=NCORES, debug=False)

    xd = nc.dram_tensor("x", [NB * H * XROW, 1], f32, kind="ExternalInput")
    iyd = nc.dram_tensor("iy", [H, NB], i32, kind="ExternalInput")
    wxd = nc.dram_tensor("wxp", [H, 2 * NB], f32, kind="ExternalInput")
    wyd = nc.dram_tensor("wyT", [NB, H, H], f32, kind="ExternalInput")
    outd = nc.dram_tensor("out", [NB, H, NCH, H], f32, kind="ExternalOutput")

    with tile.TileContext(nc) as tc:
        with (
            tc.tile_pool(name="pp", bufs=1) as ppool,
            tc.tile_pool(name="p", bufs=3) as pool,
            tc.tile_pool(name="ps", bufs=2, space="PSUM") as psum,
        ):
            iyt_all = ppool.tile([H, NB], i32, tag="iyt")
            nc.sync.dma_start(iyt_all[:], iyd.ap())
            wxt_all = ppool.tile([H, 2 * NB], f32, tag="wxt")
            nc.sync.dma_start(wxt_all[:], wxd.ap())

            for b in range(NB):
                iyt = iyt_all[:, b : b + 1]
                wxt = wxt_all[:, 2 * b : 2 * b + 2]
                wyt = pool.tile([H, H], f32, tag="wyt")
                nc.sync.dma_start(wyt[:], wyd.ap()[b])

                g = pool.tile([H, XROW], f32, tag="g")
                nc.gpsimd.indirect_dma_start(
                    out=g[:, 0:RUN],
                    out_offset=None,
                    in_=xd.ap()[:, :],
                    in_offset=bass.IndirectOffsetOnAxis(ap=iyt[:, :], axis=0),
                )

                z = psum.tile([H, XROW], f32, tag="z")
                for c0 in range(0, RUN, MMCHUNK):
                    c1 = min(c0 + MMCHUNK, RUN)
                    nc.tensor.matmul(
                        out=z[:, c0:c1],
                        lhsT=wyt[:],
                        rhs=g[:, c0:c1],
                        start=True,
                        stop=True,
                    )

                zv = z[:].rearrange("p (c w) -> p c w", w=WS)
                p1 = pool.tile([H, NCH, H], f32, tag="p1")
                nc.scalar.mul(p1[:], zv[:, :, 0:H], wxt[:, 0:1])
                ot = pool.tile([H, NCH, H], f32, tag="ot")
                nc.vector.scalar_tensor_tensor(
                    out=ot[:],
                    in0=zv[:, :, 1 : H + 1],
                    scalar=wxt[:, 1:2],
                    in1=p1[:],
                    op0=mult,
                    op1=add,
                )
                nc.sync.dma_start(outd.ap()[b], ot[:])

    nc.compile()
    return nc


def _get_program():
    if "nc" not in _PROG_CACHE:
        _PROG_CACHE["nc"] = _build_program()
    return _PROG_CACHE["nc"]


# ----------------------------------------------------------------------------
# entry point
# ----------------------------------------------------------------------------
def kernel(x, mean, var, eps, noise):
    from concourse.bass_utils import run_bass_kernel_spmd

    x = np.ascontiguousarray(np.asarray(x, np.float32))
    params = _host_params(mean, var, eps, noise)
    in_maps = [_core_inputs(x, *params, k) for k in range(NCORES)]

    nc = _get_program()
    res = run_bass_kernel_spmd(nc, in_maps, core_ids=list(range(NCORES)))
    out = np.concatenate(
        [res.results[k]["out"].transpose(0, 2, 1, 3) for k in range(NCORES)], axis=0
    )
    return np.ascontiguousarray(out.astype(np.float32))
